# revision 7
# baseline (speedup 1.0000x reference)
"""Distributed Trainium2 Bass kernel for perceiver-style cross-attention.

Reference computation (per batch element b of 64):
    query = q[b] @ Wq                      # (128, 1024)
    k, v  = split(kv[b] @ Wkv, 2)          # (512, 1024) each
    per head h (16 heads, dim 64):
        S_h = (q_h @ k_h^T) / 8            # (128, 512)
        P_h = softmax(S_h, axis=-1)
        O_h = P_h @ v_h                    # (128, 64)
    out[b] = concat_h(O_h) @ Wo + bo       # (128, 512)

Sharding: pure data-parallel over the 64-asset batch axis -> 8 assets per
NeuronCore, no collectives.

Per-core dataflow (layouts chosen so the TensorEngine contracts over
partitions and softmax needs no cross-partition reduction):
  - q/kv are PE-transposed on chip (identity matmul).
  - Projections run in f32r (fp32 storage, full-rate matmul) with N=512.
  - Scores are computed transposed, scoresT[j, i]: lhsT = kT slice,
    rhs = queryT slice (bf16, K=64).
  - exp(x/8) on ScalarE straight out of PSUM into bf16; no max subtraction
    (|scores|/8 < 8 for this problem's data, verified offline).
  - PV uses v natural with a ones column appended, so the softmax
    denominators drop out of the same matmul (row 64 of the PSUM tile).
  - Normalization after PV: out_aug * (1/s) with the reciprocal row
    broadcast across partitions by GPSIMD; final projections run at the
    end, decoupled from the per-asset loop so the normalize chain never
    stalls the PE.
  - Final projection contracts over HIDDEN with Wo natural; bias added via
    a K=1 ones-row matmul.
"""

import sys
import numpy as np

for _p in ("/opt/trn_rl_repo", "/opt/pypackages"):
    if _p not in sys.path:
        sys.path.append(_p)

from contextlib import ExitStack

import concourse.bass as bass  # noqa: E402
import concourse.tile as tile  # noqa: E402
from concourse import bacc, mybir  # noqa: E402

F32 = mybir.dt.float32
F32R = mybir.dt.float32r
BF16 = mybir.dt.bfloat16

N_CORES = 8
B_LOC = 8  # assets per core
I = 128  # num_latents
J = 512  # window size
QD = 512  # q feature dim
KVD = 256  # kv feature dim
H = 16  # heads
D = 64  # head dim
HID = 1024  # H * D
NO = 512  # output dim


def build_nc():
    nc = bacc.Bacc(
        "TRN2", target_bir_lowering=False, debug=False, num_devices=N_CORES
    )

    q_ext = nc.dram_tensor("q", [B_LOC, I, QD], F32, kind="ExternalInput").ap()
    kv_ext = nc.dram_tensor("kv", [B_LOC, J, KVD], F32, kind="ExternalInput").ap()
    wq_ext = nc.dram_tensor("Wq", [QD, HID], F32, kind="ExternalInput").ap()
    wkv_ext = nc.dram_tensor("Wkv", [KVD, 2 * HID], F32, kind="ExternalInput").ap()
    wo_ext = nc.dram_tensor("Wo", [HID, NO], F32, kind="ExternalInput").ap()
    bo_ext = nc.dram_tensor("bo", [NO], F32, kind="ExternalInput").ap()
    out_ext = nc.dram_tensor("out", [B_LOC, I, NO], F32, kind="ExternalOutput").ap()

    ident_dram = nc.inline_tensor(np.eye(128, dtype=np.float32), name="ident")

    with tile.TileContext(nc) as tc, ExitStack() as ctx:
        consts = ctx.enter_context(tc.tile_pool(name="consts", bufs=1))

        ident = consts.tile([128, 128], F32)
        nc.sync.dma_start(ident, ident_dram.ap())
        ones_f32 = consts.tile([1, 128], F32)
        nc.vector.memset(ones_f32, 1.0)
        ones_row = consts.tile([1, 128], F32R)
        nc.vector.tensor_copy(ones_row, ones_f32)

        # Persistent pools (allocated before transient phase-0 pools so pool
        # release keeps stack order).
        kvT_pool = ctx.enter_context(tc.tile_pool(name="kvTp", bufs=1))
        queryT_pool = ctx.enter_context(tc.tile_pool(name="queryTp", bufs=1))
        l_pool = ctx.enter_context(tc.tile_pool(name="lp", bufs=1))

        kvT = [
            [
                kvT_pool.tile([128, J], F32R, name=f"kvT{a}_{c}")
                for c in range(2)
            ]
            for a in range(B_LOC)
        ]
        # queryT: one [128, B*I] bf16 tile per head-PAIR (2 heads stacked on
        # partitions; base-64 operand slices are legal, HW-verified).
        queryT = [
            queryT_pool.tile([128, B_LOC * I], BF16, name=f"queryT{hc}")
            for hc in range(8)
        ]
        # normalized out^T chunks, 2 per asset-group -> 16, consumed by the
        # final projections at the end
        lgs = [
            [
                l_pool.tile([128, 4, I], F32R, name=f"lg{a}_{g}", tag=f"lg{a}_{g}")
                for g in range(2)
            ]
            for a in range(B_LOC)
        ]

        # ---------------- phase 0: input transposes + Q projection ---------
        ph0 = ExitStack()
        qnat_pool = ph0.enter_context(tc.tile_pool(name="qnat", bufs=2))
        kvnat_pool = ph0.enter_context(tc.tile_pool(name="kvnat", bufs=2))
        tps_pool = ph0.enter_context(tc.tile_pool(name="tps", bufs=4, space="PSUM"))
        qT_pool = ph0.enter_context(tc.tile_pool(name="qTp", bufs=1))
        qproj_ps_pool = ph0.enter_context(
            tc.tile_pool(name="qproj_ps", bufs=2, space="PSUM")
        )

        qT = [qT_pool.tile([128, B_LOC * I], F32R, name=f"qT{c}") for c in range(4)]

        for a in range(B_LOC):
            q_nat = qnat_pool.tile([128, QD], F32, name="q_nat")
            nc.sync.dma_start(q_nat, q_ext[a])
            for c in range(4):
                pt = tps_pool.tile([128, 128], F32, name="pt", tag="pt")
                nc.tensor.transpose(pt, q_nat[:, c * 128 : (c + 1) * 128], ident)
                nc.vector.tensor_copy(qT[c][:, a * I : (a + 1) * I], pt)
            kv_nat = kvnat_pool.tile([128, 4, KVD], F32, name="kv_nat")
            nc.sync.dma_start(
                kv_nat, kv_ext[a].rearrange("(jc p) c -> p jc c", p=128)
            )
            for c in range(2):
                for jc in range(4):
                    pt = tps_pool.tile([128, 128], F32, name="pt", tag="pt")
                    nc.tensor.transpose(
                        pt, kv_nat[:, jc, c * 128 : (c + 1) * 128], ident
                    )
                    nc.vector.tensor_copy(
                        kvT[a][c][:, jc * 128 : (jc + 1) * 128], pt
                    )

        # Weights land as f32 in temp tiles (on the scalar-engine DMA queue so
        # they never block the input DMAs on the sync queue), then are rounded
        # to f32r by a DVE copy (walrus requires f32r matmul operands to be
        # produced as f32r). Temps are released after the cast.
        wtmp = ExitStack()
        wtmp_pool = wtmp.enter_context(tc.tile_pool(name="wtmp", bufs=2))

        def _load_f32r(name, shape, src):
            tmp = wtmp_pool.tile(list(shape), F32, name=f"{name}_tmp", tag="wt")
            nc.scalar.dma_start(tmp, src)
            t = consts.tile(list(shape), F32R, name=name)
            nc.vector.tensor_copy(t, tmp)
            return t

        wq_sb = [
            _load_f32r(f"wq{c}", [128, HID], wq_ext[c * 128 : (c + 1) * 128, :])
            for c in range(4)
        ]
        wkv_sb = [
            _load_f32r(
                f"wkv{c}", [128, 2 * HID], wkv_ext[c * 128 : (c + 1) * 128, :]
            )
            for c in range(2)
        ]
        wo_sb = [
            _load_f32r(f"wo{c}", [128, NO], wo_ext[c * 128 : (c + 1) * 128, :])
            for c in range(8)
        ]
        bo_sb = _load_f32r("bo_sb", [1, NO], bo_ext.unsqueeze(0))
        wtmp.close()

        # Q projection: queryT[hd, (a, i)] = sum_c Wq[c, hd] qT[c, (a, i)]
        for hc in range(8):
            for nh in range(2):
                ps = qproj_ps_pool.tile([128, 512], F32, name="qps", tag="qps")
                for cc in range(4):
                    nc.tensor.matmul(
                        ps,
                        wq_sb[cc][:, hc * 128 : (hc + 1) * 128],
                        qT[cc][:, nh * 512 : (nh + 1) * 512],
                        start=(cc == 0),
                        stop=(cc == 3),
                    )
                nc.vector.tensor_copy(
                    queryT[hc][:, nh * 512 : (nh + 1) * 512], ps
                )

        ph0.close()

        # ---------------- per-asset attention pipeline ---------------------
        proj_ps_pool = ctx.enter_context(
            tc.tile_pool(name="proj_ps", bufs=2, space="PSUM")
        )
        score_ps_pool = ctx.enter_context(
            tc.tile_pool(name="score_ps", bufs=2, space="PSUM")
        )
        aug_ps_pool = ctx.enter_context(
            tc.tile_pool(name="aug_ps", bufs=2, space="PSUM")
        )
        kT_pool = ctx.enter_context(tc.tile_pool(name="kTp", bufs=2))
        v_pool = ctx.enter_context(tc.tile_pool(name="vp", bufs=2))
        exp_pool = ctx.enter_context(tc.tile_pool(name="expp", bufs=3))
        s_pool = ctx.enter_context(tc.tile_pool(name="sp", bufs=2))
        rb_pool = ctx.enter_context(tc.tile_pool(name="rbp", bufs=2))
        o_pool = ctx.enter_context(tc.tile_pool(name="op", bufs=2))

        for a in range(B_LOC):
            # K projection, transposed: kT[hd, j], one tile per head pair
            kT = [
                kT_pool.tile([128, J], BF16, name=f"kT{hc}", tag=f"kT{hc}")
                for hc in range(8)
            ]
            for hc in range(8):
                ps = proj_ps_pool.tile([128, J], F32, name="pps", tag="pps")
                for cc in range(2):
                    nc.tensor.matmul(
                        ps,
                        wkv_sb[cc][:, hc * 128 : (hc + 1) * 128],
                        kvT[a][cc],
                        start=(cc == 0),
                        stop=(cc == 1),
                    )
                nc.vector.tensor_copy(kT[hc], ps)

            # V projection, natural: v[j, hd] with a ones column per head
            vaug = v_pool.tile([128, 4, H, D + 1], BF16, name="vaug", tag="vaug")
            nc.vector.memset(vaug[:, :, :, D : D + 1], 1.0)
            for jc in range(4):
                for nh in range(2):
                    ps = proj_ps_pool.tile([128, 512], F32, name="pps", tag="pps")
                    for cc in range(2):
                        nc.tensor.matmul(
                            ps,
                            kvT[a][cc][:, jc * 128 : (jc + 1) * 128],
                            wkv_sb[cc][:, HID + nh * 512 : HID + (nh + 1) * 512],
                            start=(cc == 0),
                            stop=(cc == 1),
                        )
                    nc.scalar.copy(
                        vaug[:, jc, nh * 8 : (nh + 1) * 8, 0:D],
                        ps.rearrange("p (h d) -> p h d", h=8),
                    )

            # attention in two groups of 8 heads
            for g in range(2):
                aug = aug_ps_pool.tile([D + 1, 8, I], F32, name="aug", tag="aug")
                for hh in range(8):
                    h = g * 8 + hh
                    hp = h % 2  # head parity within its pair tile
                    sps = score_ps_pool.tile([128, 4, I], F32, name="sps", tag="sps")
                    for jc in range(4):
                        nc.tensor.matmul(
                            sps[:, jc, :],
                            kT[h // 2][
                                hp * D : (hp + 1) * D, jc * 128 : (jc + 1) * 128
                            ],
                            queryT[h // 2][
                                hp * D : (hp + 1) * D, a * I : (a + 1) * I
                            ],
                            start=True,
                            stop=True,
                        )
                    expT = exp_pool.tile([128, 4, I], BF16, name="expT", tag="expT")
                    nc.scalar.activation(
                        expT,
                        sps,
                        mybir.ActivationFunctionType.Exp,
                        bias=0.0,
                        scale=0.125,
                    )
                    for jc in range(4):
                        nc.tensor.matmul(
                            aug[:, hh, :],
                            vaug[:, jc, h, :],
                            expT[:, jc, :],
                            start=(jc == 0),
                            stop=(jc == 3),
                        )

                # normalize: reciprocal of the s row, broadcast, multiply
                srow = s_pool.tile([1, 8 * I], F32, name="srow", tag="srow")
                nc.scalar.copy(srow, aug[D : D + 1, :, :])
                s8 = s_pool.tile([8, I], F32, name="s8", tag="s8")
                nc.sync.dma_start(s8, srow)
                r8 = s_pool.tile([8, I], F32, name="r8", tag="r8")
                nc.vector.reciprocal(r8, s8)
                rrow = s_pool.tile([1, 8 * I], F32, name="rrow", tag="rrow")
                nc.sync.dma_start(rrow, r8)
                rb = rb_pool.tile([128, 8, I], F32, name="rb", tag="rb")
                nc.gpsimd.partition_broadcast(rb[:], rrow[:])

                lg = lgs[a][g]
                # even local heads -> partitions 0:64, odd -> 64:128
                nc.vector.tensor_mul(
                    lg[0:64, :, :], aug[0:64, 0:8:2, :], rb[0:64, 0:8:2, :]
                )
                nc.vector.tensor_mul(
                    lg[64:96, :, :], aug[0:32, 1:8:2, :], rb[0:32, 1:8:2, :]
                )
                nc.vector.tensor_mul(
                    lg[96:128, :, :], aug[32:64, 1:8:2, :], rb[32:64, 1:8:2, :]
                )

        # ---------------- final projections (decoupled from asset loop) ----
        for a in range(B_LOC):
            fps = score_ps_pool.tile([128, NO], F32, name="sps", tag="sps")
            for g in range(2):
                for cc in range(4):
                    c = 4 * g + cc
                    nc.tensor.matmul(
                        fps,
                        lgs[a][g][:, cc, :],
                        wo_sb[c],
                        start=(c == 0),
                        stop=False,
                    )
            nc.tensor.matmul(fps, ones_row, bo_sb, start=False, stop=True)
            out_sb = o_pool.tile([128, NO], F32, name="out_sb", tag="out_sb")
            nc.scalar.copy(out_sb, fps)
            nc.sync.dma_start(out_ext[a], out_sb)

    nc.compile()
    return nc


_CACHED_NC = None


def kernel(q, kv, Wq, Wkv, Wo, bo):
    global _CACHED_NC
    from concourse.bass_utils import run_bass_kernel_spmd

    if _CACHED_NC is None:
        _CACHED_NC = build_nc()
    nc = _CACHED_NC

    q = np.ascontiguousarray(np.asarray(q, dtype=np.float32))
    kv = np.ascontiguousarray(np.asarray(kv, dtype=np.float32))
    Wq = np.ascontiguousarray(np.asarray(Wq, dtype=np.float32))
    Wkv = np.ascontiguousarray(np.asarray(Wkv, dtype=np.float32))
    Wo = np.ascontiguousarray(np.asarray(Wo, dtype=np.float32))
    bo = np.ascontiguousarray(np.asarray(bo, dtype=np.float32))

    in_maps = []
    for c in range(N_CORES):
        sl = slice(c * B_LOC, (c + 1) * B_LOC)
        in_maps.append(
            {"q": q[sl], "kv": kv[sl], "Wq": Wq, "Wkv": Wkv, "Wo": Wo, "bo": bo}
        )

    res = run_bass_kernel_spmd(nc, in_maps, list(range(N_CORES)))
    out = np.concatenate(
        [res.results[c]["out"].reshape(B_LOC, I, NO) for c in range(N_CORES)],
        axis=0,
    )
    return out
